# revision 1
# baseline (speedup 1.0000x reference)
"""Collective-free sequence-sharded causal self-attention for 8 TRN2 cores.

Sharding: core c -> batch b = c//2, zig-zag half z = c%2.  The core computes
ALL 16 heads for two 512-token query blocks of its batch:
    block A: queries [512z, 512z+512)        (kv extent 1024, mask_a)
    block B: queries [2048-512(z+1), ...+512) (kv extent 2048; kv<1024 is
             fully causal -> unmasked; kv in [1024,2048) uses mask_b)
The zig-zag pairing makes every core's instruction graph IDENTICAL (SPMD)
while balancing true causal work; causality differences live in per-core
mask DATA supplied by the host.  K/V for the batch's first 2048 tokens are
recomputed on both cores of a pair (cheaper than any collective here).

Everything runs bf16 with fp32 PSUM accumulation; softmax normalizer via a
ones column appended per head to V (row 64 of the AV PSUM = sum_k P); no
max-subtraction (logits are O(5)).  y^T stays in SBUF; the projection for
the core's own tokens reads it directly.  Output is the core's 1024 token
rows of the final [8192, 1024], reassembled by the host.
"""

import numpy as np
import ml_dtypes

import concourse.bass as bass
import concourse.mybir as mybir
import concourse.tile as tile
from concourse import bacc
from concourse import bass_utils
from concourse.masks import make_identity

F32 = mybir.dt.float32
BF16 = mybir.dt.bfloat16

B, T, C = 4, 2048, 1024
NH, HS = 16, 64
NCORES = 8
NTOK = B * T
P = 128
KO = C // P                 # 8 contraction chunks over C
QW = 512                    # query block width
KC_W = 128                  # kv chunk width (PSUM partition)
GRP = 2                     # kv chunks per exp group
KV_A, KV_B = 1024, 2048     # kv extents of block A / block B
NFC = NH // 2               # 8 feature chunks of 128 (2 heads each)


def build_graph():
    nc = bacc.Bacc(
        "TRN2",
        target_bir_lowering=False,
        debug=False,
        enable_asserts=True,
        num_devices=NCORES,
    )

    xq = nc.dram_tensor("xq", [C, 2 * QW], BF16, kind="ExternalInput").ap()
    xkv = nc.dram_tensor("xkv", [C, KV_B], BF16, kind="ExternalInput").ap()
    w_qkv = nc.dram_tensor("w_qkv", [C, 3 * C], BF16, kind="ExternalInput").ap()
    b_qkv = nc.dram_tensor("b_qkv", [3 * C], F32, kind="ExternalInput").ap()
    w_proj = nc.dram_tensor("w_proj", [C, C], BF16, kind="ExternalInput").ap()
    b_proj = nc.dram_tensor("b_proj", [C], F32, kind="ExternalInput").ap()
    mask_a = nc.dram_tensor("mask_a", [KV_A, QW], BF16, kind="ExternalInput").ap()
    mask_b = nc.dram_tensor("mask_b", [KV_A, QW], BF16, kind="ExternalInput").ap()
    out = nc.dram_tensor("out", [2 * QW, C], F32, kind="ExternalOutput").ap()

    xq_t = xq.rearrange("(ko p) t -> p ko t", p=P)        # [128, 8, 1024]
    xkv_t = xkv.rearrange("(ko p) t -> p ko t", p=P)      # [128, 8, 2048]
    wq_t = w_qkv.rearrange("(ko p) f -> p ko f", p=P)     # [128, 8, 3072]
    wp_t = w_proj.rearrange("(ko p) f -> p ko f", p=P)    # [128, 8, 1024]
    ma_t = mask_a.rearrange("(kc p) q -> p kc q", p=P)    # [128, 8, 512]
    mb_t = mask_b.rearrange("(kc p) q -> p kc q", p=P)    # [128, 8, 512]

    with tile.TileContext(nc) as tc:
        with (
            tc.tile_pool(name="const", bufs=1) as const,
            tc.tile_pool(name="w", bufs=1) as w_pool,
            tc.tile_pool(name="xslab", bufs=2) as xslab_pool,
            tc.tile_pool(name="qk", bufs=1) as qk_pool,
            tc.tile_pool(name="vtok", bufs=1) as v_pool,
            tc.tile_pool(name="pexp", bufs=6) as p_pool,
            tc.tile_pool(name="small", bufs=2) as small_pool,
            tc.tile_pool(name="outsb", bufs=2) as out_pool,
            tc.tile_pool(name="mm_ps", bufs=2, space="PSUM") as mm_ps,
            tc.tile_pool(name="st_ps", bufs=2, space="PSUM") as st_ps,
            tc.tile_pool(name="y_ps", bufs=2, space="PSUM") as y_ps,
        ):
            # ---- QKV pass 1a: q^T for block A only ----
            qT = qk_pool.tile([P, NFC, 2 * QW], BF16, tag="qT")
            w_q = w_pool.tile([P, KO, C], BF16, tag="w", name="w_q")
            for fq in range(NFC):
                nc.sync.dma_start(w_q[:, :, fq * P:(fq + 1) * P],
                                  wq_t[:, :, fq * P:(fq + 1) * P])

            # ---- small constants ----
            bqk_sb = const.tile([P, 24], F32)      # qkv bias, per-partition
            nc.sync.dma_start(bqk_sb[:], b_qkv.rearrange("(c p) -> p c", p=P))
            bp_row = const.tile([1, C], F32)
            nc.sync.dma_start(bp_row[:], b_proj[None, :])
            ones_row = const.tile([1, P], BF16)
            nc.vector.memset(ones_row[:], 1.0)
            ones_col = const.tile([P, KV_B // P, 1], F32)
            nc.vector.memset(ones_col[:], 1.0)

            # v bias broadcast across token partitions: [1,1024] -> [128,1024]
            bv_row = const.tile([1, C], F32)
            nc.sync.dma_start(bv_row[:], b_qkv[None, 2 * C:])
            bv_row16 = const.tile([1, C], BF16)
            nc.vector.tensor_copy(bv_row16[:], bv_row[:])
            bv_bc = const.tile([P, C], F32)
            for half in range(2):
                bv_ps = mm_ps.tile([P, QW], F32, tag="mm")
                nc.tensor.matmul(bv_ps[:], ones_row[:],
                                 bv_row16[:, half * QW:(half + 1) * QW],
                                 start=True, stop=True)
                nc.vector.tensor_copy(bv_bc[:, half * QW:(half + 1) * QW], bv_ps[:])


            def qt_slab(s, w_tile):
                slab = xslab_pool.tile([P, KO, QW], BF16, tag="xslab",
                                       name=f"xq{s}")
                for kd in range(KO):
                    nc.sync.dma_start(slab[:, kd, :],
                                      xq_t[:, kd, s * QW:(s + 1) * QW])
                for f in range(NFC):
                    ps = mm_ps.tile([P, QW], F32, tag="mm")
                    for k0 in range(KO):
                        nc.tensor.matmul(
                            ps[:], w_tile[:, k0, f * P:(f + 1) * P],
                            slab[:, k0, :],
                            start=(k0 == 0), stop=(k0 == KO - 1),
                        )
                    nc.scalar.activation(
                        qT[:, f, s * QW:(s + 1) * QW], ps[:],
                        mybir.ActivationFunctionType.Identity,
                        bias=bqk_sb[:, f:f + 1],
                    )

            qt_slab(0, w_q)
            qt_slab(1, w_q)

            # ---- QKV pass 2: k^T and v over 2048 kv tokens ----
            kT = qk_pool.tile([P, NFC, KV_B], BF16, tag="kT")
            v_aug = v_pool.tile([P, KV_B // P, NH * (HS + 1)], BF16, tag="v")
            for h in range(NH):
                nc.vector.tensor_copy(
                    v_aug[:, :, h * (HS + 1) + HS:h * (HS + 1) + HS + 1],
                    ones_col[:])
            w_kv = w_pool.tile([P, KO, 2 * C], BF16, tag="w", name="w_kv")
            for fq in range(2 * NFC):
                nc.sync.dma_start(w_kv[:, :, fq * P:(fq + 1) * P],
                                  wq_t[:, :, C + fq * P:C + (fq + 1) * P])

            def kv_slab(s):
                slab = xslab_pool.tile([P, KO, QW], BF16, tag="xslab",
                                       name=f"xkv{s}")
                for kd in range(KO):
                    nc.sync.dma_start(slab[:, kd, :],
                                      xkv_t[:, kd, s * QW:(s + 1) * QW])
                for f in range(NFC):
                    ps = mm_ps.tile([P, QW], F32, tag="mm")
                    for k0 in range(KO):
                        nc.tensor.matmul(
                            ps[:], w_kv[:, k0, f * P:(f + 1) * P],
                            slab[:, k0, :],
                            start=(k0 == 0), stop=(k0 == KO - 1),
                        )
                    nc.scalar.activation(
                        kT[:, f, s * QW:(s + 1) * QW], ps[:],
                        mybir.ActivationFunctionType.Identity,
                        bias=bqk_sb[:, 8 + f:9 + f],
                    )
                # v token-major directly: lhsT = x slab, rhs = W_v half
                for t4 in range(QW // P):
                    tc_i = s * (QW // P) + t4
                    for nn in range(2):
                        ps = mm_ps.tile([P, QW], F32, tag="mm")
                        for k0 in range(KO):
                            nc.tensor.matmul(
                                ps[:], slab[:, k0, t4 * P:(t4 + 1) * P],
                                w_kv[:, k0, C + nn * QW:C + (nn + 1) * QW],
                                start=(k0 == 0), stop=(k0 == KO - 1),
                            )
                        for hh in range(8):
                            h = 8 * nn + hh
                            nc.vector.tensor_tensor(
                                v_aug[:, tc_i, h * (HS + 1):h * (HS + 1) + HS],
                                ps[:, hh * HS:(hh + 1) * HS],
                                bv_bc[:, h * HS:(h + 1) * HS],
                                mybir.AluOpType.add)

            kv_slab(0)
            kv_slab(1)
            # masks can load behind the first kv slabs
            ma_sb = const.tile([P, KV_A // P, QW], BF16)
            nc.sync.dma_start(ma_sb[:], ma_t)
            mb_sb = const.tile([P, KV_A // P, QW], BF16)
            nc.sync.dma_start(mb_sb[:], mb_t)

            # ---- attention: head pairs packed per feature chunk ----
            # For each fc f, heads e=2f (partitions 0:64) and o=2f+1 (64:128)
            # issue S^T matmuls into slabs j=0/1 of one [128,2,512] PSUM tile
            # with tile_position (0,0)/(64,0) -> concurrent PE row-groups.
            # One exp + one (broadcast) mask op covers both heads.
            yT = qk_pool.tile([P, NFC, 2 * QW], BF16, tag="yT")  # y^T local

            def attn_block(f, blk, nkc, masked_from):
                yps_e = y_ps.tile([P, QW], F32, tag="y", name=f"y_e_{f}_{blk}")
                yps_o = y_ps.tile([P, QW], F32, tag="y", name=f"y_o_{f}_{blk}")
                for kc in range(nkc):
                    stps = st_ps.tile([P, 2, QW], F32, tag="st")
                    for j, hp in ((0, 0), (1, HS)):
                        nc.tensor.matmul(
                            stps[:, j, :],
                            kT[hp:hp + HS, f, kc * KC_W:(kc + 1) * KC_W],
                            qT[hp:hp + HS, f, blk * QW:(blk + 1) * QW],
                            start=True, stop=True,
                            tile_position=(hp, 0),
                        )
                    pexp = p_pool.tile([P, 2, QW], BF16, tag="p")
                    nc.scalar.activation(
                        pexp[:], stps[:],
                        mybir.ActivationFunctionType.Exp,
                        scale=1.0 / np.sqrt(HS),
                    )
                    if blk == 0 or kc >= masked_from:
                        m2 = ma_sb if blk == 0 else mb_sb
                        kcm = kc if blk == 0 else kc - masked_from
                        nc.vector.tensor_tensor(
                            pexp[:], pexp[:],
                            m2[:, kcm:kcm + 1, :].to_broadcast((P, 2, QW)),
                            mybir.AluOpType.mult)
                    for j, h in ((0, 2 * f), (1, 2 * f + 1)):
                        vc = h * (HS + 1)
                        yps = yps_e if j == 0 else yps_o
                        nc.tensor.matmul(
                            yps[0:HS + 1, :],
                            v_aug[:, kc, vc:vc + HS + 1],
                            pexp[:, j, :],
                            start=(kc == 0), stop=(kc == nkc - 1),
                        )
                for j, hp in ((0, 0), (1, HS)):
                    yps = yps_e if j == 0 else yps_o
                    # evict to SBUF right away so the PSUM bank frees for the
                    # next head pair; normalize from SBUF afterwards
                    y_sb = small_pool.tile([HS + 1, QW], F32, tag="y_sb")
                    nc.vector.tensor_copy(y_sb[:], yps[0:HS + 1, :])
                    recip = small_pool.tile([1, QW], BF16, tag="recip")
                    with nc.allow_low_precision(
                            reason="bf16 softmax normalizer within tolerance"):
                        nc.vector.reciprocal(recip[:], y_sb[HS:HS + 1, :])
                    bcps = mm_ps.tile([HS, QW], F32, tag="mm")
                    nc.tensor.matmul(bcps[:], ones_row[:, :HS], recip[:],
                                     start=True, stop=True)
                    bc_sb = small_pool.tile([HS, QW], F32, tag="bc_sb")
                    nc.vector.tensor_copy(bc_sb[:], bcps[:])
                    nc.vector.tensor_tensor(
                        yT[hp:hp + HS, f, blk * QW:(blk + 1) * QW],
                        y_sb[0:HS, :], bc_sb[:], mybir.AluOpType.mult,
                    )

            # block A only needs kv slabs 0-1: interleave its head pairs with
            # the remaining kv production and the block-B q^T pass
            for f in range(3):
                attn_block(f, 0, KV_A // KC_W, KV_A // KC_W)
            kv_slab(2)
            for f in range(3, 6):
                attn_block(f, 0, KV_A // KC_W, KV_A // KC_W)
            kv_slab(3)
            for f in range(6, NFC):
                attn_block(f, 0, KV_A // KC_W, KV_A // KC_W)

            # ---- projection for own 1024 tokens (all output columns) ----
            def proj_chunks(tms):
                for tm in tms:
                    for nn in range(C // QW):  # 2 column chunks of 512
                        ps = mm_ps.tile([P, QW], F32, tag="mm")
                        for k0 in range(KO):
                            nc.tensor.matmul(
                                ps[:],
                                yT[:, k0, tm * P:(tm + 1) * P],
                                w_p[:, k0, nn * QW:(nn + 1) * QW],
                                start=(k0 == 0), stop=(k0 == KO - 1),
                            )
                        osb = out_pool.tile([P, QW], F32, tag="osb")
                        nc.vector.tensor_tensor(
                            osb[:], ps[:], bp_bc[:, nn * QW:(nn + 1) * QW],
                            mybir.AluOpType.add,
                        )
                        nc.sync.dma_start(
                            out[tm * P:(tm + 1) * P, nn * QW:(nn + 1) * QW],
                            osb[:])

            # proj bias broadcast across partitions: [1,1024] -> [128,1024]
            bp_row16 = const.tile([1, C], BF16)
            nc.vector.tensor_copy(bp_row16[:], bp_row[:])
            bp_bc = const.tile([P, C], F32)
            for half in range(2):
                bp_ps = mm_ps.tile([P, QW], F32, tag="mm")
                nc.tensor.matmul(bp_ps[:], ones_row[:],
                                 bp_row16[:, half * QW:(half + 1) * QW],
                                 start=True, stop=True)
                nc.vector.tensor_copy(bp_bc[:, half * QW:(half + 1) * QW], bp_ps[:])

            w_p = w_pool.tile([P, KO, C], BF16, tag="w", name="w_p")
            nc.sync.dma_start(w_p[:], wp_t)
            proj_chunks(range(0, 4))           # block A rows, overlaps B-attn
            for f in range(NFC):
                attn_block(f, 1, KV_B // KC_W, KV_A // KC_W)
            proj_chunks(range(4, 8))           # block B rows

    nc.compile()
    return nc


_NC_CACHE = None


def _get_nc():
    global _NC_CACHE
    if _NC_CACHE is None:
        _NC_CACHE = build_graph()
    return _NC_CACHE


def _q_ranges(c):
    """Global token rows (within [0, 8192)) of core c's blocks A and B."""
    b, z = c // 2, c % 2
    a0 = b * T + 512 * z
    b0 = b * T + T - 512 * (z + 1)
    return (a0, a0 + QW), (b0, b0 + QW)


def make_in_maps(x, W_attn, b_attn, W_proj, b_proj):
    x = np.asarray(x, dtype=np.float32)
    W_attn = np.asarray(W_attn, dtype=np.float32)
    b_attn = np.asarray(b_attn, dtype=np.float32)
    W_proj = np.asarray(W_proj, dtype=np.float32)
    b_proj = np.asarray(b_proj, dtype=np.float32)

    bf = ml_dtypes.bfloat16
    xT = np.ascontiguousarray(x.reshape(NTOK, C).T).astype(bf)  # [1024, 8192]
    wq = np.ascontiguousarray(W_attn).astype(bf)
    wp = np.ascontiguousarray(W_proj).astype(bf)
    kv = np.arange(KV_B)

    in_maps = []
    for c in range(NCORES):
        b, z = c // 2, c % 2
        (a0, a1), (b0, b1) = _q_ranges(c)
        xq_c = np.ascontiguousarray(
            np.concatenate([xT[:, a0:a1], xT[:, b0:b1]], axis=1))
        xkv_c = np.ascontiguousarray(xT[:, b * T:b * T + KV_B])
        qa = np.arange(a0 - b * T, a1 - b * T)   # q positions within batch
        qb = np.arange(b0 - b * T, b1 - b * T)
        m_a = (kv[:KV_A, None] <= qa[None, :]).astype(bf)           # [1024,512]
        m_b = (kv[KV_A:, None] <= qb[None, :]).astype(bf)           # [1024,512]
        in_maps.append({
            "xq": xq_c, "xkv": xkv_c,
            "w_qkv": wq, "b_qkv": b_attn,
            "w_proj": wp, "b_proj": b_proj,
            "mask_a": np.ascontiguousarray(m_a),
            "mask_b": np.ascontiguousarray(m_b),
        })
    return in_maps


def kernel(x, W_attn, b_attn, W_proj, b_proj):
    nc = _get_nc()
    in_maps = make_in_maps(x, W_attn, b_attn, W_proj, b_proj)
    res = bass_utils.run_bass_kernel_spmd(
        nc, in_maps, core_ids=list(range(NCORES)), trace=False,
    )
    out_full = np.empty((NTOK, C), dtype=np.float32)
    for c in range(NCORES):
        (a0, a1), (b0, b1) = _q_ranges(c)
        blk = res.results[c]["out"]
        out_full[a0:a1] = blk[:QW]
        out_full[b0:b1] = blk[QW:]
    kernel.last_results = res
    return out_full.reshape(B, T, C)



# revision 4
# speedup vs baseline: 1.3877x; 1.3877x over previous
"""Collective-free causal self-attention, v2: 4-slot zig-zag on 8 TRN2 cores.

Sharding: core c -> batch b = c//2, zig-zag half z = c%2.  The core computes
ALL 16 heads for FOUR 256-token query slots of its batch, with kv extents
512/1024/1536/2048 (identical instruction graph on every core; causality and
the per-core query assignment live in mask DATA and a host-side kv token
permutation).  Slot query ranges (local to the batch):
    z=0: [0,256) [512,768) [1280,1536) [1792,2048)
    z=1: [256,512) [768,1024) [1024,1280) [1536,1792)
Each 512-token kv slab is permuted so the core's own slot queries occupy its
first 256 columns; q^T is therefore computed from the kv slabs directly (no
separate xq input).  Only the last 4 kv chunks of each slot carry a mask
(diagonal band); earlier chunks are provably fully-causal for both cores of
the pair.

Everything runs bf16 with fp32 PSUM accumulation; softmax normalizer via a
ones column appended per head to V (row 64 of the AV PSUM = sum_k P); no
max-subtraction (logits are O(5)).  Engine balance: PE matmuls, ACT exp only,
DVE masks/normalize/qk-bias evictions, Pool v/proj bias evictions.  Output is
the core's 1024 token rows (slot-major) of the final [8192, 1024].
"""

import numpy as np
import ml_dtypes

import concourse.bass as bass
import concourse.mybir as mybir
import concourse.tile as tile
from concourse import bacc
from concourse import bass_utils

F32 = mybir.dt.float32
BF16 = mybir.dt.bfloat16

B, T, C = 4, 2048, 1024
NH, HS = 16, 64
VW = HS + 1                 # v feature width incl. ones column
NCORES = 8
NTOK = B * T
P = 128
KO = C // P                 # 8 contraction chunks over C
SW = 512                    # kv slab width
QB = 256                    # slot query width
NSLOT = 4
NFC = NH // 2               # 8 feature chunks of 128 (2 heads each)
EXT = (4, 8, 12, 16)        # kv extent of each slot, in 128-chunks
Q_STARTS = ((0, 512, 1280, 1792), (256, 768, 1024, 1536))  # per z


def build_graph(repeat=1):
    nc = bacc.Bacc(
        "TRN2",
        target_bir_lowering=False,
        debug=False,
        enable_asserts=True,
        num_devices=NCORES,
    )

    xkv = nc.dram_tensor("xkv", [C, T], BF16, kind="ExternalInput").ap()
    w_qkv = nc.dram_tensor("w_qkv", [C, 3 * C], BF16, kind="ExternalInput").ap()
    b_qkv = nc.dram_tensor("b_qkv", [3 * C], F32, kind="ExternalInput").ap()
    w_proj = nc.dram_tensor("w_proj", [C, C], BF16, kind="ExternalInput").ap()
    b_proj = nc.dram_tensor("b_proj", [C], F32, kind="ExternalInput").ap()
    mask = nc.dram_tensor("mask", [4 * SW, QB], BF16, kind="ExternalInput").ap()
    out = nc.dram_tensor("out", [NSLOT * QB, C], F32, kind="ExternalOutput").ap()

    xkv_t = xkv.rearrange("(ko p) t -> p ko t", p=P)      # [128, 8, 2048]
    wq_t = w_qkv.rearrange("(ko p) f -> p ko f", p=P)     # [128, 8, 3072]
    wp_t = w_proj.rearrange("(ko p) f -> p ko f", p=P)    # [128, 8, 1024]
    mk_t = mask.rearrange("(m p) q -> p m q", p=P)        # [128, 16, 256]

    with tile.TileContext(nc) as tc:
        with (
            tc.tile_pool(name="const", bufs=1) as const,
            tc.tile_pool(name="w", bufs=1) as w_pool,
            tc.tile_pool(name="xslab", bufs=2) as xslab_pool,
            tc.tile_pool(name="qk", bufs=1) as qk_pool,
            tc.tile_pool(name="vtok", bufs=1) as v_pool,
            tc.tile_pool(name="pexp", bufs=6) as p_pool,
            tc.tile_pool(name="small", bufs=3) as small_pool,
            tc.tile_pool(name="outsb", bufs=2) as out_pool,
            tc.tile_pool(name="mm_ps", bufs=2, space="PSUM") as mm_ps,
            tc.tile_pool(name="st_ps", bufs=2, space="PSUM") as st_ps,
            tc.tile_pool(name="y_ps", bufs=2, space="PSUM") as y_ps,
        ):
            for _rep in range(repeat):
                # ---- small constants (DMAs issued first, tiny) ----
                bqk_sb = const.tile([P, 24], F32)
                nc.sync.dma_start(bqk_sb[:], b_qkv.rearrange("(c p) -> p c", p=P))
                bv_row = const.tile([1, C], F32)
                nc.sync.dma_start(bv_row[:], b_qkv[None, 2 * C:])
                bp_row = const.tile([1, C], F32)
                nc.sync.dma_start(bp_row[:], b_proj[None, :])

                # ---- persistent tiles ----
                w_all = w_pool.tile([P, KO, 3 * C], BF16, tag="w", name="w_all")
                w_p = w_pool.tile([P, KO, C], BF16, tag="w", name="w_p")
                qT = qk_pool.tile([P, NFC, NSLOT * QB], BF16, tag="qT")
                kT = qk_pool.tile([P, NFC, T], BF16, tag="kT")
                v_aug = v_pool.tile([P, T // P, NH, VW], BF16, tag="v")
                ma_sb = const.tile([P, 16, 1, QB], BF16)
                yT = qk_pool.tile([P, NFC, NSLOT * QB], BF16, tag="yT")

                # first slab's x, then q/k weights, mask, v weights
                slab0 = xslab_pool.tile([P, KO, SW], BF16, tag="xslab",
                                        name="x0")
                for kd in range(KO):
                    nc.sync.dma_start(slab0[:, kd, :],
                                      xkv_t[:, kd, 0:SW])
                for fq in range(2 * NFC):          # q then k weight chunks
                    nc.sync.dma_start(w_all[:, :, fq * P:(fq + 1) * P],
                                      wq_t[:, :, fq * P:(fq + 1) * P])
                nc.sync.dma_start(ma_sb[:, :, 0, :], mk_t)
                for fq in range(2 * NFC, 3 * NFC):  # v weight chunks
                    nc.sync.dma_start(w_all[:, :, fq * P:(fq + 1) * P],
                                      wq_t[:, :, fq * P:(fq + 1) * P])

                # ones column of v_aug (Pool), ones row for broadcasts
                ones_row = const.tile([1, P], BF16)
                nc.vector.memset(ones_row[:], 1.0)
                nc.gpsimd.memset(v_aug[:, :, :, HS:HS + 1], 1.0)

                # v/proj bias broadcast across partitions via ones matmul
                bv_row16 = const.tile([1, C], BF16)
                nc.gpsimd.tensor_copy(bv_row16[:], bv_row[:])
                bp_row16 = const.tile([1, C], BF16)
                nc.gpsimd.tensor_copy(bp_row16[:], bp_row[:])
                bv_bc = const.tile([P, 2, NFC, HS], BF16)
                bp_bc = const.tile([P, C], BF16)
                for half in range(2):
                    ps = mm_ps.tile([P, SW], F32, tag="mm")
                    nc.tensor.matmul(ps[:], ones_row[:],
                                     bv_row16[:, half * SW:(half + 1) * SW],
                                     start=True, stop=True)
                    nc.vector.tensor_copy(
                        bv_bc[:, half, :, :],
                        ps[:].rearrange("p (h w) -> p h w", h=NFC))
                    ps2 = mm_ps.tile([P, SW], F32, tag="mm")
                    nc.tensor.matmul(ps2[:], ones_row[:],
                                     bp_row16[:, half * SW:(half + 1) * SW],
                                     start=True, stop=True)
                    nc.vector.tensor_copy(bp_bc[:, half * SW:(half + 1) * SW],
                                          ps2[:])

                def x_dma(s):
                    slab_t = xslab_pool.tile([P, KO, SW], BF16,
                                             tag="xslab", name=f"x{s}")
                    for kd in range(KO):
                        nc.sync.dma_start(slab_t[:, kd, :],
                                          xkv_t[:, kd, s * SW:(s + 1) * SW])
                    return slab_t

                def q_gen(s, slab_t):
                    # q^T for slot s: own queries are the slab's first QB cols
                    for f in range(NFC):
                        ps = mm_ps.tile([P, QB], F32, tag="mm")
                        for k0 in range(KO):
                            nc.tensor.matmul(
                                ps[:], w_all[:, k0, f * P:(f + 1) * P],
                                slab_t[:, k0, 0:QB],
                                start=(k0 == 0), stop=(k0 == KO - 1),
                            )
                        nc.vector.tensor_scalar_add(
                            qT[:, f, s * QB:(s + 1) * QB], ps[:],
                            bqk_sb[:, f:f + 1])
                        yield

                def kv_gen(s, slab_t):
                    # k^T for the whole slab
                    for f in range(NFC):
                        ps = mm_ps.tile([P, SW], F32, tag="mm")
                        for k0 in range(KO):
                            nc.tensor.matmul(
                                ps[:], w_all[:, k0, C + f * P:C + (f + 1) * P],
                                slab_t[:, k0, :],
                                start=(k0 == 0), stop=(k0 == KO - 1),
                            )
                        nc.vector.tensor_scalar_add(
                            kT[:, f, s * SW:(s + 1) * SW], ps[:],
                            bqk_sb[:, 8 + f:9 + f])
                        yield
                    # v token-major
                    for t4 in range(SW // P):
                        tc_i = s * (SW // P) + t4
                        for nn in range(2):
                            ps = mm_ps.tile([P, SW], F32, tag="mm")
                            for k0 in range(KO):
                                nc.tensor.matmul(
                                    ps[:], slab_t[:, k0, t4 * P:(t4 + 1) * P],
                                    w_all[:, k0, 2 * C + nn * SW:
                                          2 * C + (nn + 1) * SW],
                                    start=(k0 == 0), stop=(k0 == KO - 1),
                                )
                            nc.vector.tensor_tensor(
                                v_aug[:, tc_i, 8 * nn:8 * nn + 8, 0:HS],
                                ps[:].rearrange("p (h w) -> p h w", h=8),
                                bv_bc[:, nn, :, :],
                                mybir.AluOpType.add)
                            yield

                def attn_slot(f, s):
                    ext = EXT[s]
                    yps_e = y_ps.tile([P, QB], F32, tag="y",
                                      name=f"y_e_{f}_{s}")
                    yps_o = y_ps.tile([P, QB], F32, tag="y",
                                      name=f"y_o_{f}_{s}")
                    for kc0 in range(0, ext, 2):
                        pexp = p_pool.tile([P, 2, 2, QB], BF16, tag="p")
                        for kci in range(2):
                            kc = kc0 + kci
                            # per-kc PSUM tile: each head's matmul output is
                            # bank-aligned (tile_position + sub-bank offset
                            # wedges the PE)
                            stps = st_ps.tile([P, 2, 2 * QB], F32, tag="st")
                            for j, hp in ((0, 0), (1, HS)):
                                nc.tensor.matmul(
                                    stps[:, j, 0:QB],
                                    kT[hp:hp + HS, f, kc * P:(kc + 1) * P],
                                    qT[hp:hp + HS, f, s * QB:(s + 1) * QB],
                                    start=True, stop=True,
                                    tile_position=(hp, 0),
                                )
                            nc.scalar.activation(
                                pexp[:, kci, :, :], stps[:, :, 0:QB],
                                mybir.ActivationFunctionType.Exp,
                                scale=1.0 / np.sqrt(HS),
                            )
                        if kc0 >= ext - 4:
                            mi = 4 * s + (kc0 - (ext - 4))
                            nc.vector.tensor_tensor(
                                pexp[:], pexp[:],
                                ma_sb[:, mi:mi + 2, :, :].to_broadcast(
                                    (P, 2, 2, QB)),
                                mybir.AluOpType.mult)
                        for kci in range(2):
                            kc = kc0 + kci
                            for j, h in ((0, 2 * f), (1, 2 * f + 1)):
                                yps = yps_e if j == 0 else yps_o
                                nc.tensor.matmul(
                                    yps[0:VW, :],
                                    v_aug[:, kc, h, :],
                                    pexp[:, kci, j, :],
                                    start=(kc == 0), stop=(kc == ext - 1),
                                )
                        yield
                    for j, hp in ((0, 0), (1, HS)):
                        yps = yps_e if j == 0 else yps_o
                        y_sb = small_pool.tile([VW, QB], BF16, tag="y_sb")
                        nc.vector.tensor_copy(y_sb[:], yps[0:VW, :])
                        recip = small_pool.tile([1, QB], BF16, tag="recip")
                        with nc.allow_low_precision(
                                reason="bf16 softmax normalizer within tol"):
                            nc.vector.reciprocal(recip[:], y_sb[HS:HS + 1, :])
                        # y-pool, not mm: keeps normalize from blocking
                        # the woven filler groups' mm slots
                        bcps = y_ps.tile([HS, QB], F32, tag="y",
                                         name=f"bc_{f}_{s}_{j}")
                        nc.tensor.matmul(bcps[:], ones_row[:, :HS], recip[:],
                                         start=True, stop=True)
                        nc.vector.tensor_tensor(
                            yT[hp:hp + HS, f, s * QB:(s + 1) * QB],
                            y_sb[0:HS, :], bcps[:], mybir.AluOpType.mult,
                        )
                        yield

                def proj(tm):
                    for nn in range(2):
                        ps = mm_ps.tile([P, SW], F32, tag="mm")
                        for k0 in range(KO):
                            nc.tensor.matmul(
                                ps[:],
                                yT[:, k0, tm * P:(tm + 1) * P],
                                w_p[:, k0, nn * SW:(nn + 1) * SW],
                                start=(k0 == 0), stop=(k0 == KO - 1),
                            )
                        osb = out_pool.tile([P, SW], F32, tag="osb")
                        nc.vector.tensor_tensor(
                            osb[:], ps[:], bp_bc[:, nn * SW:(nn + 1) * SW],
                            mybir.AluOpType.add)
                        nc.sync.dma_start(
                            out[tm * P:(tm + 1) * P, nn * SW:(nn + 1) * SW],
                            osb[:])
                        yield

                # ---- woven schedule: slot s's first pairs only need
                # slabs < s, so slab s (and later proj) interleave INTO the
                # attention stream, giving PE filler work during exp waits --
                def drain(g):
                    for _ in g:
                        pass

                def weave(attn_gens, fillers):
                    steps = sum(EXT[s] // 2 + 2 for _, s in attn_gens)
                    fsteps = len(fillers)
                    credit = 0.0
                    gens = [attn_slot(f, s) for f, s in attn_gens]
                    for g in gens:
                        for _ in g:
                            credit += fsteps / steps
                            while credit >= 1.0 and fillers:
                                try:
                                    next(fillers[0])
                                    credit -= 1.0
                                except StopIteration:
                                    fillers.pop(0)
                    for fg in fillers:
                        drain(fg)

                x1 = x_dma(1)
                drain(q_gen(0, slab0))
                drain(kv_gen(0, slab0))
                # slot-major phases: slab s+1 production woven into slot s,
                # projections woven into slot 3
                weave([(f, 0) for f in range(NFC)],
                      [q_gen(1, x1)] * 8 + [kv_gen(1, x1)] * 16)
                x2 = x_dma(2)
                nc.sync.dma_start(w_p[:], wp_t)
                weave([(f, 1) for f in range(NFC)],
                      [q_gen(2, x2)] * 8 + [kv_gen(2, x2)] * 16)
                x3 = x_dma(3)
                weave([(f, 2) for f in range(NFC)],
                      [q_gen(3, x3)] * 8 + [kv_gen(3, x3)] * 16)
                weave([(f, 3) for f in range(NFC)],
                      [g for g in [proj(tm) for tm in range(6)]
                       for _ in range(2)])
                drain(proj(6))
                drain(proj(7))

    nc.compile()
    return nc


_NC_CACHE = None


def _get_nc():
    global _NC_CACHE
    if _NC_CACHE is None:
        _NC_CACHE = build_graph()
    return _NC_CACHE


def _perm(z):
    """Permuted local token index (within the batch) for each of the 2048
    kv slab positions, and the slot query ranges."""
    starts = Q_STARTS[z]
    perm = []
    for s in range(NSLOT):
        lo, hi = s * SW, (s + 1) * SW
        own = np.arange(starts[s], starts[s] + QB)
        other = np.array([t for t in range(lo, hi)
                          if not (starts[s] <= t < starts[s] + QB)])
        perm.append(np.concatenate([own, other]))
    return np.concatenate(perm)      # [2048]


def make_in_maps(x, W_attn, b_attn, W_proj, b_proj):
    x = np.asarray(x, dtype=np.float32)
    W_attn = np.asarray(W_attn, dtype=np.float32)
    b_attn = np.asarray(b_attn, dtype=np.float32)
    W_proj = np.asarray(W_proj, dtype=np.float32)
    b_proj = np.asarray(b_proj, dtype=np.float32)

    bf = ml_dtypes.bfloat16
    xT = np.ascontiguousarray(x.reshape(NTOK, C).T).astype(bf)  # [1024, 8192]
    wq = np.ascontiguousarray(W_attn).astype(bf)
    wp = np.ascontiguousarray(W_proj).astype(bf)

    in_maps = []
    for c in range(NCORES):
        b, z = c // 2, c % 2
        perm = _perm(z)
        xkv_c = np.ascontiguousarray(xT[:, b * T + perm])
        starts = Q_STARTS[z]
        masks = []
        for s in range(NSLOT):
            E = EXT[s] * P
            rows = perm[E - SW:E]
            q_local = starts[s] + np.arange(QB)
            masks.append((rows[:, None] <= q_local[None, :]).astype(bf))
        mask_c = np.ascontiguousarray(np.concatenate(masks, axis=0))
        in_maps.append({
            "xkv": xkv_c,
            "w_qkv": wq, "b_qkv": b_attn,
            "w_proj": wp, "b_proj": b_proj,
            "mask": mask_c,
        })
    return in_maps


def kernel(x, W_attn, b_attn, W_proj, b_proj):
    nc = _get_nc()
    in_maps = make_in_maps(x, W_attn, b_attn, W_proj, b_proj)
    res = bass_utils.run_bass_kernel_spmd(
        nc, in_maps, core_ids=list(range(NCORES)), trace=False,
    )
    out_full = np.empty((NTOK, C), dtype=np.float32)
    for c in range(NCORES):
        b, z = c // 2, c % 2
        starts = Q_STARTS[z]
        blk = res.results[c]["out"]
        for s in range(NSLOT):
            g0 = b * T + starts[s]
            out_full[g0:g0 + QB] = blk[s * QB:(s + 1) * QB]
    kernel.last_results = res
    return out_full.reshape(B, T, C)
